# revision 19
# baseline (speedup 1.0000x reference)
"""Trainium2 Bass kernel for nn_BestRqLossNetwork (best-RQ masked-prediction loss).

Math (per the reference):
    logits  = context @ W_enc + b_enc                      # (N,T,K)
    targets = argmin_k ||normalize(feats @ proj) - cb_k||  # == argmax_k (feats@proj)·cb_k
                                                           #    (cb rows unit-norm, row norm > 0)
    loss    = mean over valid (t < lens[n]) of CE(logits, targets)

Distribution: data-parallel over the 8192 (n, t) positions — 1024 consecutive
tokens per core (each core's slab lies inside one sequence since T = 2*1024).
Weights (W_enc, codebook, proj) are replicated. Each core returns its local
(sum_nll, valid_count); the host sums the 16 scalars and divides.

Per-core pipeline, per 128-token tile (tokens on partitions):
  PE   : scores = fT8.T @ cbt8 — both replicated 8x along the contraction so
         all 128 PE rows are active (HAM keeps the clock-gate at 8/8; with
         16-row scores matmuls interleaved the PE sat at 4/8 = 1.2 GHz for
         the whole steady state). The 8x score scale is a power of two:
         argmax and bf16 rounding are unaffected.
         logits = ctxT.T @ W (fp8 DoubleRow) into 1024-wide PSUM groups.
  ACT  : exp with row-sum accumulation (logsumexp without max subtraction:
         |logits| <= ~6 so exp cannot overflow). One deferred Ln at the end.
         The scalar queue is left empty so ACT only ever runs exps.
  DVE  : fused PSUM->SBUF copy + per-1024-chunk max (tensor_scalar accum) —
         the only full-K touch DVE pays (any second pass or shaped reduce
         costs the same ~1.2ns/elem again); two-level argmax (MAX_INDEX over
         the 8 chunk maxes -> indirect-DMA gather of the winning 1024-chunk
         from a DRAM staging buffer -> MAX_INDEX within it).
  Pool : all index arithmetic, the valid-count mask, and the target-logit
         dot(ctx_row, gathered W_enc.T row) — keeps DVE under the PE's phase
         time so the scores PSUM never backpressures the PE.

Scheduling: engines execute their streams IN ORDER; emission is a uniform
software pipeline (phase j = logits(j) + scores(j+1)) with one tile-phase of
latency cover for each staging store -> gather -> consume hop:
  pre:     scores(0) + chainA(0)
  phase j: logits(j) groups interleaved with scores(j+1) megas;
           chainB(j) spread over early groups (L2 argmax + W-row gather);
           dot(j-1) at g=6; chainA(j+1) at the end (L1 argmax + score-chunk
           gather issue).
A few warm-up matmuls on zeroed SBUF run at the very start so the PE's HAM
clock-gate reaches 2.4 GHz before the real work arrives.
"""

import os
import numpy as np
import ml_dtypes

_GPS_IDX = os.environ.get("V4_GPS_IDX", "1") == "1"
# Pool scalar_tensor_tensor fails walrus codegen — keep the dot on DVE.
_GPS_DOT = os.environ.get("V4_GPS_DOT", "0") == "1"
_GPS_MASK = os.environ.get("V4_GPS_MASK", "1") == "1"
_GPS_DOT2 = os.environ.get("V4_GPS_DOT2", "0") == "1"

N, T, F, V, K = 4, 2048, 512, 16, 8192
NCORES = 8
TOK = (N * T) // NCORES   # tokens per core
P = 128                   # partitions / tokens per tile
NTILES = TOK // P         # 8
CC = F // P               # 4 contraction chunks of 128
MC = K // 1024            # 8 mega-chunks of 1024 classes

_BF16 = ml_dtypes.bfloat16
_FP8 = ml_dtypes.float8_e4m3
_cache: dict = {}


def build_program(has_bias: bool):
    """Build + compile the single-core Bass program (run SPMD on 8 cores)."""
    from concourse import bacc
    import concourse.bass as bass
    import concourse.tile as tile
    import concourse.mybir as mybir

    dt = mybir.dt
    alu = mybir.AluOpType
    act = mybir.ActivationFunctionType

    nc = bacc.Bacc(
        "TRN2", target_bir_lowering=False, debug=False, num_devices=NCORES
    )

    ctxT = nc.dram_tensor("ctxT", [F, TOK], dt.float8e4, kind="ExternalInput").ap()
    ctx = nc.dram_tensor("ctx", [TOK, F], dt.bfloat16, kind="ExternalInput").ap()
    featsT = nc.dram_tensor("featsT", [F, TOK], dt.bfloat16, kind="ExternalInput").ap()
    w = nc.dram_tensor("w", [F, K], dt.float8e4, kind="ExternalInput").ap()
    wt = nc.dram_tensor("wt", [K, F], dt.bfloat16, kind="ExternalInput").ap()
    # codebook.T replicated 8x along the contraction dim (row g*16+v = cb[:,v])
    cbt8 = nc.dram_tensor("cbt8", [P, K], dt.bfloat16, kind="ExternalInput").ap()
    # proj replicated 8x along its output dim (col g*16+v = proj[:,v])
    proj8 = nc.dram_tensor("proj8", [F, P], dt.bfloat16, kind="ExternalInput").ap()
    adjlen = nc.dram_tensor("adjlen", [P, 1], dt.float32, kind="ExternalInput").ap()
    tidx = nc.dram_tensor("tidx", [P, 1], dt.float32, kind="ExternalInput").ap()
    tidx_i = nc.dram_tensor("tidx_i", [P, 1], dt.int32, kind="ExternalInput").ap()
    if has_bias:
        brow = nc.dram_tensor("brow", [1, K], dt.bfloat16, kind="ExternalInput").ap()
        bcol = nc.dram_tensor("bcol", [K, 1], dt.float32, kind="ExternalInput").ap()
    out2 = nc.dram_tensor("out2", [2, 1], dt.float32, kind="ExternalOutput").ap()
    # DRAM staging for the two-level argmax: row (tok*MC + mc) holds that
    # token's mc-th 1024-wide score chunk (bf16).
    stage = nc.dram_tensor("scstage", [TOK * MC, 1024], dt.bfloat16).ap()
    stage_v = stage.rearrange("(t m) k -> t m k", m=MC)

    with tile.TileContext(nc) as tc:
        with (
            tc.tile_pool(name="singles", bufs=1) as singles,
            tc.tile_pool(name="work", bufs=3) as work,
            tc.tile_pool(name="stg", bufs=6) as stg,
            tc.tile_pool(name="sc_ps", bufs=2, space="PSUM") as sc_ps_pool,
            tc.tile_pool(name="lg_ps", bufs=2, space="PSUM") as lg_ps_pool,
        ):
            # ---- resident SBUF tensors ----
            w_sb = singles.tile([P, CC, K], dt.float8e4)
            ctxT_sb = singles.tile([P, CC, TOK], dt.float8e4)
            featsT_sb = singles.tile([P, CC, TOK], dt.bfloat16)
            ctx_sb = singles.tile([P, NTILES, F], dt.bfloat16)
            cbt8_sb = singles.tile([P, K], dt.bfloat16)
            proj8_sb = singles.tile([P, CC, P], dt.bfloat16)
            fT8_sb = singles.tile([P, TOK], dt.bfloat16)
            adjlen_sb = singles.tile([P, 1], dt.float32)
            tidx_sb = singles.tile([P, 1], dt.float32)
            tidxi_sb = singles.tile([P, 1], dt.int32)
            ones_sb = singles.tile([P, 1], dt.float32)
            warm_sb = singles.tile([P, 512], dt.bfloat16)
            exp_scr = singles.tile([P, 1024], dt.bfloat16)
            dot_scr = singles.tile([P, F], dt.bfloat16)
            nll_all = singles.tile([P, NTILES], dt.float32)
            cnt_all = singles.tile([P, NTILES], dt.float32)
            s_all = singles.tile([P, NTILES], dt.float32)
            lt_all = singles.tile([P, NTILES], dt.float32)
            logs_all = singles.tile([P, NTILES], dt.float32)
            stack2 = singles.tile([P, 2], dt.float32)
            out_sb = singles.tile([2, 1], dt.float32)

            # PE warm-up: matmuls on zeroed SBUF with no DMA dependency keep
            # the PE busy from t=0 so the HAM clock-gate opens to 2.4 GHz
            # while the input DMAs stream in.
            nc.vector.memset(warm_sb[:, :], 0.0)
            def emit_warm_mm(n=1):
                for _ in range(n):
                    wz = sc_ps_pool.tile([P, 1024], dt.float32, tag="sp", name="wz")
                    nc.tensor.matmul(
                        out=wz[:, 0:512], lhsT=warm_sb[:, 0:P], rhs=warm_sb[:, :],
                        start=True, stop=True,
                    )

            emit_warm_mm(24)

            # Startup loads. The sync queue is reserved for the per-chunk
            # score staging DMAs; the scalar queue's issues all complete
            # before the first exp (logits(0) runs in phase 0, after the
            # load window), so ACT is never delayed in the steady state.
            for cc in range(CC):
                nc.scalar.dma_start(out=featsT_sb[:, cc, :], in_=featsT[cc * P:(cc + 1) * P, :])
            # W in per-mega-chunk slices so the first logits matmul can start
            # after ~1 MB instead of the full 8 MB. Issue time is split
            # between the scalar queue (idle until tile 0's exps) and the
            # sync queue (ahead of the staging stores, which are first
            # consumed by the chainA(0) gather at loop j=1) so neither
            # queue eats the full ~20us of descriptor generation.
            for g in range(MC):
                q = nc.scalar if g < MC // 2 else nc.sync
                for cc in range(CC):
                    q.dma_start(
                        out=w_sb[:, cc, g * 1024:(g + 1) * 1024],
                        in_=w[cc * P:(cc + 1) * P, g * 1024:(g + 1) * 1024],
                    )
            for cc in range(CC):
                nc.gpsimd.dma_start(out=proj8_sb[:, cc, :], in_=proj8[cc * P:(cc + 1) * P, :])
            # cbt8 in K-halves so the first scores mega can start early
            nc.gpsimd.dma_start(out=cbt8_sb[:, 0:K // 2], in_=cbt8[:, 0:K // 2])
            for cc in range(CC):
                nc.gpsimd.dma_start(out=ctxT_sb[:, cc, :], in_=ctxT[cc * P:(cc + 1) * P, :])
            nc.gpsimd.dma_start(out=cbt8_sb[:, K // 2:], in_=cbt8[:, K // 2:])
            for j in range(NTILES):
                nc.gpsimd.dma_start(out=ctx_sb[:, j, :], in_=ctx[j * P:(j + 1) * P, :])
            nc.gpsimd.dma_start(out=adjlen_sb[:, :], in_=adjlen[:, :])
            nc.gpsimd.dma_start(out=tidx_sb[:, :], in_=tidx[:, :])
            nc.gpsimd.dma_start(out=tidxi_sb[:, :], in_=tidx_i[:, :])
            nc.vector.memset(ones_sb[:, :], 1.0)

            if has_bias:
                onesrow_sb = singles.tile([1, P], dt.bfloat16)
                brow_sb = singles.tile([1, K], dt.bfloat16)
                nc.vector.memset(onesrow_sb[:, :], 1.0)
                nc.gpsimd.dma_start(out=brow_sb[:, :], in_=brow[:, :])

            # ---- fT8 = ((feats @ proj).T replicated 8x) : (128, TOK), bf16 ----
            fT8_ps = lg_ps_pool.tile([P, TOK], dt.float32, tag="lp")
            for h in range(TOK // 512):
                for cc in range(CC):
                    nc.tensor.matmul(
                        out=fT8_ps[:, h * 512:(h + 1) * 512],
                        lhsT=proj8_sb[:, cc, :],
                        rhs=featsT_sb[:, cc, h * 512:(h + 1) * 512],
                        start=(cc == 0),
                        stop=(cc == CC - 1),
                    )
            nc.vector.tensor_copy(out=fT8_sb[:, :], in_=fT8_ps[:, :])

            # ---- software-pipelined main loop ----
            st = {}  # per-tile live tiles: cmA/m1/mc8/scrow/widx/wrow

            def emit_scores_mega(t, mc):
                """One 1024-wide scores mega-chunk: two full-contraction
                matmuls into one PSUM tile, a single fused copy+max, then DMA
                the chunk to DRAM."""
                tsl = slice(t * P, (t + 1) * P)
                s = st.setdefault(t, {})
                if mc == 0:
                    s["cmA"] = work.tile([P, MC], dt.float32, tag="cma", name=f"cma{t}", bufs=4)
                if mc % 2 == 0:
                    # one staging tile covers two megas -> one coalesced DMA
                    s["sstg"] = stg.tile([P, 2, 1024], dt.bfloat16, tag="sstg",
                                         name=f"sstg{t}_{mc}")
                sstg = s["sstg"]
                sp = sc_ps_pool.tile([P, 1024], dt.float32, tag="sp")
                for h in range(2):
                    nc.tensor.matmul(
                        out=sp[:, h * 512:(h + 1) * 512],
                        lhsT=fT8_sb[:, tsl],
                        rhs=cbt8_sb[:, mc * 1024 + h * 512:mc * 1024 + (h + 1) * 512],
                        start=True,
                        stop=True,
                    )
                nc.vector.tensor_scalar(
                    out=sstg[:, mc % 2, :],
                    in0=sp[:, :],
                    scalar1=0.0,
                    scalar2=None,
                    op0=alu.add,
                    op1=alu.max,
                    accum_out=s["cmA"][:, mc:mc + 1],
                )
                if mc % 2 == 1:
                    nc.sync.dma_start(out=stage_v[tsl, mc - 1:mc + 1, :],
                                      in_=sstg[:, :, :])

            def emit_chainA(t):
                """Level-1 argmax over chunk maxes (DVE) + row index math and
                the score-chunk gather issue (Pool)."""
                s = st[t]
                cm = s["cmA"]
                m1 = work.tile([P, 1], dt.float32, tag="m1", name=f"m1_{t}")
                nc.vector.tensor_reduce(
                    out=m1[:, :], in_=cm[:, :], axis=mybir.AxisListType.X, op=alu.max
                )
                m8 = work.tile([P, 8], dt.float32, tag="m8", name=f"m8_{t}")
                nc.vector.tensor_copy(out=m8[:, :], in_=m1[:, 0:1].to_broadcast([P, 8]))
                mc8 = work.tile([P, 8], dt.uint32, tag="mc8", name=f"mc8_{t}")
                nc.vector.max_index(mc8[:, :], m8[:, :], cm[:, :])
                rowid = work.tile([P, 1], dt.int32, tag="rowid", name=f"rid{t}")
                _eng_idx = nc.gpsimd if _GPS_IDX else nc.vector
                _eng_idx.tensor_scalar(
                    out=rowid[:, :], in0=tidxi_sb[:, :],
                    scalar1=float(MC), scalar2=float(t * P * MC),
                    op0=alu.mult, op1=alu.add,
                )
                _eng_idx.tensor_tensor(
                    out=rowid[:, :], in0=rowid[:, :],
                    in1=mc8[:, 0:1].bitcast(dt.int32), op=alu.add,
                )
                scrow = work.tile([P, 1024], dt.bfloat16, tag="scrow",
                                  name=f"scrow{t}")
                nc.gpsimd.indirect_dma_start(
                    out=scrow[:, :],
                    out_offset=None,
                    in_=stage[:, :],
                    in_offset=bass.IndirectOffsetOnAxis(ap=rowid[:, 0:1], axis=0),
                )
                s["m1"], s["mc8"], s["scrow"] = m1, mc8, scrow

            def emit_chainB(t, part):
                """Level-2 argmax within the gathered chunk (staged a full
                tile-phase ago) + the W_enc.T row gather. Split into parts so
                the DVE/Pool streams interleave with the per-mega work."""
                s = st[t]
                if part == 0:
                    m1b = work.tile([P, 1], dt.bfloat16, tag="m1b", name=f"m1b{t}")
                    nc.vector.tensor_copy(out=m1b[:, :], in_=s["m1"][:, :])
                    m8b = work.tile([P, 8], dt.bfloat16, tag="m8b", name=f"m8b{t}")
                    nc.vector.tensor_copy(out=m8b[:, :], in_=m1b[:, 0:1].to_broadcast([P, 8]))
                    s["m8b"] = m8b
                elif part == 1:
                    l2i = work.tile([P, 8], dt.uint32, tag="l2i", name=f"l2i{t}")
                    nc.vector.max_index(l2i[:, :], s["m8b"][:, :], s["scrow"][:, :])
                    s["l2i"] = l2i
                elif part == 2:
                    widx = work.tile([P, 1], dt.int32, tag="widx", name=f"widx{t}")
                    _eng_idx = nc.gpsimd if _GPS_IDX else nc.vector
                    _eng_idx.tensor_scalar(
                        out=widx[:, :], in0=s["mc8"][:, 0:1].bitcast(dt.int32),
                        scalar1=1024.0, scalar2=None, op0=alu.mult,
                    )
                    _eng_idx.tensor_tensor(
                        out=widx[:, :], in0=widx[:, :],
                        in1=s["l2i"][:, 0:1].bitcast(dt.int32), op=alu.add,
                    )
                    s["widx"] = widx
                elif part == 3:
                    wrow = work.tile([P, F], dt.bfloat16, tag="wrow", name=f"wrow{t}")
                    nc.gpsimd.indirect_dma_start(
                        out=wrow[:, :],
                        out_offset=None,
                        in_=wt[:, :],
                        in_offset=bass.IndirectOffsetOnAxis(ap=s["widx"][:, 0:1], axis=0),
                    )
                    s["wrow"] = wrow
                    if has_bias:
                        bg = work.tile([P, 1], dt.float32, tag="bg", name=f"bg{t}")
                        nc.gpsimd.indirect_dma_start(
                            out=bg[:, :],
                            out_offset=None,
                            in_=bcol[:, :],
                            in_offset=bass.IndirectOffsetOnAxis(ap=s["widx"][:, 0:1], axis=0),
                        )
                        s["bg"] = bg

            def emit_dot(t):
                """Target logit via dot(ctx_row, W_row) (gather landed during
                the previous tile phase)."""
                s = st[t]
                if _GPS_DOT2:
                    # Pool two-step: elementwise product, then accumulate.
                    nc.gpsimd.tensor_tensor(
                        out=dot_scr[:, :], in0=ctx_sb[:, t, :],
                        in1=s["wrow"][:, :], op=alu.mult,
                    )
                    nc.gpsimd.tensor_scalar(
                        out=dot_scr[:, :], in0=dot_scr[:, :],
                        scalar1=1.0, scalar2=None, op0=alu.mult, op1=alu.add,
                        accum_out=lt_all[:, t:t + 1],
                    )
                    if has_bias:
                        nc.gpsimd.tensor_add(
                            lt_all[:, t:t + 1], lt_all[:, t:t + 1], s["bg"][:, :]
                        )
                    del st[t]
                    return
                (nc.gpsimd if _GPS_DOT else nc.vector).scalar_tensor_tensor(
                    out=dot_scr[:, :],
                    in0=ctx_sb[:, t, :],
                    scalar=1.0,
                    in1=s["wrow"][:, :],
                    op0=alu.mult,
                    op1=alu.mult,
                    accum_out=lt_all[:, t:t + 1],
                )
                if has_bias:
                    nc.gpsimd.tensor_add(
                        lt_all[:, t:t + 1], lt_all[:, t:t + 1], s["bg"][:, :]
                    )
                del st[t]

            def emit_logits_group(j, g, sums):
                tsl = slice(j * P, (j + 1) * P)
                lp = lg_ps_pool.tile([P, 1024], dt.float32, tag="lp")
                for h in range(2):
                    hsl = slice(h * 512, (h + 1) * 512)
                    for cc2 in range(0, CC, 2):
                        nc.tensor.matmul(
                            out=lp[:, hsl],
                            lhsT=ctxT_sb[:, cc2:cc2 + 2, tsl],
                            rhs=w_sb[:, cc2:cc2 + 2, g * 1024 + h * 512:g * 1024 + (h + 1) * 512],
                            start=(cc2 == 0),
                            stop=(cc2 == CC - 2 and not has_bias),
                            perf_mode=mybir.MatmulPerfMode.DoubleRow,
                        )
                    if has_bias:
                        nc.tensor.matmul(
                            out=lp[:, hsl],
                            lhsT=onesrow_sb[:, :],
                            rhs=brow_sb[:, g * 1024 + h * 512:g * 1024 + (h + 1) * 512],
                            start=False,
                            stop=True,
                        )
                nc.scalar.activation(
                    out=exp_scr[:, :],
                    in_=lp[:, :],
                    func=act.Exp,
                    scale=1.0 / 64.0,
                    accum_out=sums[:, g:g + 1],
                )

            def emit_mask(j):
                # valid mask: (tidx - adjlen) < -128*j  <=>  j*128 + tidx < len - t_off
                (nc.gpsimd if _GPS_MASK else nc.vector).tensor_scalar(
                    out=cnt_all[:, j:j + 1],
                    in0=tidx_sb[:, :],
                    scalar1=adjlen_sb[:, 0:1],
                    scalar2=float(-(j * P)),
                    op0=alu.subtract,
                    op1=alu.is_lt,
                )

            # Prologue: scores for tiles 0-2 (DVE-paced — there is no logits
            # work to hide them behind yet) interleaved with tile 0's logits
            # groups, which start as soon as their W slices land. The steady
            # drip of matmuls keeps the HAM clock-gate warm through the whole
            # input-load window.
            sums0 = work.tile([P, MC], dt.float32, tag="sums", name="sums0")
            for g in range(MC):
                for t in range(3):
                    emit_scores_mega(t, g)
                emit_logits_group(0, g, sums0)
            nc.vector.tensor_reduce(
                out=s_all[:, 0:1], in_=sums0[:, :],
                axis=mybir.AxisListType.X, op=alu.add,
            )
            emit_mask(0)
            emit_chainA(0)

            for j in range(1, NTILES):
                if j >= 2:
                    emit_dot(j - 2)
                for part in range(4):
                    emit_chainB(j - 1, part)
                if j < NTILES - 1:
                    emit_chainA(j)
                    if j == NTILES - 2:
                        emit_chainA(NTILES - 1)
                else:
                    for part in range(4):
                        emit_chainB(NTILES - 1, part)

                # logits (PE) + exp/row-sum (ACT), with tile j+2's scores
                # matmuls interleaved between groups
                sums = work.tile([P, MC], dt.float32, tag="sums", name=f"sums{j}")
                for g in range(MC):
                    if 3 <= j + 2 < NTILES:
                        emit_scores_mega(j + 2, g)
                    if j == NTILES - 1:
                        if g == 3:
                            emit_dot(j - 1)
                        elif g == 6:
                            emit_dot(j)
                    emit_logits_group(j, g, sums)

                nc.vector.tensor_reduce(
                    out=s_all[:, j:j + 1], in_=sums[:, :],
                    axis=mybir.AxisListType.X, op=alu.add,
                )
                emit_mask(j)

            # ---- epilogue: one Ln for all tiles (avoids per-tile ACT
            # table-set switches between Exp and Ln), nll assembly, then the
            # partition reduction via ones-matmul ----
            nc.scalar.activation(out=logs_all[:, :], in_=s_all[:, :], func=act.Ln)
            nc.vector.tensor_sub(nll_all[:, :], logs_all[:, :], lt_all[:, :])
            nc.vector.tensor_mul(nll_all[:, :], nll_all[:, :], cnt_all[:, :])
            nc.vector.tensor_reduce(
                out=stack2[:, 0:1], in_=nll_all[:, :], axis=mybir.AxisListType.X,
                op=alu.add,
            )
            nc.vector.tensor_reduce(
                out=stack2[:, 1:2], in_=cnt_all[:, :], axis=mybir.AxisListType.X,
                op=alu.add,
            )
            fin_ps = sc_ps_pool.tile([2, 1], dt.float32, tag="sp")
            nc.tensor.matmul(
                out=fin_ps[:, :], lhsT=stack2[:, :], rhs=ones_sb[:, :],
                start=True, stop=True,
            )
            nc.vector.tensor_copy(out=out_sb[:, :], in_=fin_ps[:, :])
            nc.sync.dma_start(out=out2[:, :], in_=out_sb[:, :])

    nc.compile()
    return nc


def _get_program(has_bias: bool):
    if has_bias not in _cache:
        _cache[has_bias] = build_program(has_bias)
    return _cache[has_bias]


def make_in_maps(feats, context, lens, proj_matrix, codebook, W_enc, b_enc,
                 has_bias):
    """Shard + lay out the full inputs into per-core input maps."""
    feats_f = np.ascontiguousarray(feats).reshape(N * T, F)
    ctx_f = np.ascontiguousarray(context).reshape(N * T, F)
    w_f8 = (W_enc * 64.0).astype(_FP8)
    wt_bf = np.ascontiguousarray(W_enc.T).astype(_BF16)
    cbt8_bf = np.ascontiguousarray(np.tile(codebook.T.astype(_BF16), (8, 1)))
    proj8_bf = np.ascontiguousarray(np.tile(proj_matrix, (1, 8))).astype(_BF16)
    tidx_a = np.arange(P, dtype=np.float32).reshape(P, 1)
    tidx_ia = np.arange(P, dtype=np.int32).reshape(P, 1)

    in_maps = []
    for c in range(NCORES):
        sl = slice(c * TOK, (c + 1) * TOK)
        ctxs = ctx_f[sl]
        featss = feats_f[sl]
        n_idx = (c * TOK) // T
        t_off = (c * TOK) % T
        adj = np.full((P, 1), float(int(lens[n_idx]) - t_off), dtype=np.float32)
        m = {
            "ctxT": np.ascontiguousarray(ctxs.T).astype(_FP8),
            "ctx": ctxs.astype(_BF16),
            "featsT": np.ascontiguousarray(featss.T).astype(_BF16),
            "w": w_f8,
            "wt": wt_bf,
            "cbt8": cbt8_bf,
            "proj8": proj8_bf,
            "adjlen": adj,
            "tidx": tidx_a,
            "tidx_i": tidx_ia,
        }
        if has_bias:
            m["brow"] = np.ascontiguousarray(b_enc * 64.0).reshape(1, K).astype(_BF16)
            m["bcol"] = np.ascontiguousarray(b_enc).reshape(K, 1).astype(np.float32)
        in_maps.append(m)
    return in_maps


def kernel(feats, context, lens, proj_matrix, codebook, W_enc, b_enc,
           _want_results=False, _trace=False):
    from concourse.bass_utils import run_bass_kernel_spmd

    has_bias = bool(np.any(np.asarray(b_enc) != 0))
    nc = _get_program(has_bias)
    in_maps = make_in_maps(feats, context, lens, proj_matrix, codebook, W_enc,
                           b_enc, has_bias)
    res = run_bass_kernel_spmd(
        nc, in_maps, list(range(NCORES)), trace=_trace,
        trace_cores=list(range(NCORES)) if _trace else None,
    )
    num = sum(float(r["out2"][0, 0]) for r in res.results)
    cnt = sum(float(r["out2"][1, 0]) for r in res.results)
    loss = np.array(np.float32(num / max(cnt, 1.0)))
    if _want_results:
        return loss, res
    return loss


if __name__ == "__main__":
    import jax
    cpu = jax.devices("cpu")[0]
    import reference

    with jax.default_device(cpu):
        inputs = reference.setup_inputs()
        inputs = {k: np.asarray(v) for k, v in inputs.items()}
        expected = float(np.asarray(reference.reference(**inputs)))
    loss = float(kernel(**inputs))
    rel = abs(loss - expected) / max(abs(expected), 1e-30)
    print(f"expected {expected} got {loss} rel {rel:.3e}")


# revision 20
# speedup vs baseline: 1.1426x; 1.1426x over previous
"""Trainium2 Bass kernel for nn_BestRqLossNetwork (best-RQ masked-prediction loss).

Math (per the reference):
    logits  = context @ W_enc + b_enc                      # (N,T,K)
    targets = argmin_k ||normalize(feats @ proj) - cb_k||  # == argmax_k (feats@proj)·cb_k
                                                           #    (cb rows unit-norm, row norm > 0)
    loss    = mean over valid (t < lens[n]) of CE(logits, targets)

Distribution: data-parallel over the 8192 (n, t) positions — 1024 consecutive
tokens per core (each core's slab lies inside one sequence since T = 2*1024).
Weights (W_enc, codebook, proj) are replicated. Each core returns its local
(sum_nll, valid_count); the host sums the 16 scalars and divides.

Per-core pipeline, per 128-token tile (tokens on partitions):
  PE   : scores = fT8.T @ cbt8 — both replicated 8x along the contraction so
         all 128 PE rows are active (HAM keeps the clock-gate at 8/8; with
         16-row scores matmuls interleaved the PE sat at 4/8 = 1.2 GHz for
         the whole steady state). The 8x score scale is a power of two:
         argmax and bf16 rounding are unaffected.
         logits = ctxT.T @ W (fp8 DoubleRow) into 1024-wide PSUM groups.
  ACT  : exp with row-sum accumulation (logsumexp without max subtraction:
         |logits| <= ~6 so exp cannot overflow). One deferred Ln at the end.
         The scalar queue is left empty so ACT only ever runs exps.
  DVE  : fused PSUM->SBUF copy + per-1024-chunk max (tensor_scalar accum) —
         the only full-K touch DVE pays (any second pass or shaped reduce
         costs the same ~1.2ns/elem again); two-level argmax (MAX_INDEX over
         the 8 chunk maxes -> indirect-DMA gather of the winning 1024-chunk
         from a DRAM staging buffer -> MAX_INDEX within it).
  Pool : all index arithmetic, the valid-count mask, and the target-logit
         dot(ctx_row, gathered W_enc.T row) — keeps DVE under the PE's phase
         time so the scores PSUM never backpressures the PE.

Scheduling: engines execute their streams IN ORDER; emission is a uniform
software pipeline (phase j = logits(j) + scores(j+1)) with one tile-phase of
latency cover for each staging store -> gather -> consume hop:
  pre:     scores(0) + chainA(0)
  phase j: logits(j) groups interleaved with scores(j+1) megas;
           chainB(j) spread over early groups (L2 argmax + W-row gather);
           dot(j-1) at g=6; chainA(j+1) at the end (L1 argmax + score-chunk
           gather issue).
A few warm-up matmuls on zeroed SBUF run at the very start so the PE's HAM
clock-gate reaches 2.4 GHz before the real work arrives.
"""

import os
import numpy as np
import ml_dtypes

_GPS_IDX = os.environ.get("V4_GPS_IDX", "1") == "1"
# Pool scalar_tensor_tensor fails walrus codegen — keep the dot on DVE.
_GPS_DOT = os.environ.get("V4_GPS_DOT", "0") == "1"
_GPS_MASK = os.environ.get("V4_GPS_MASK", "1") == "1"
_GPS_DOT2 = os.environ.get("V4_GPS_DOT2", "0") == "1"

N, T, F, V, K = 4, 2048, 512, 16, 8192
NCORES = 8
TOK = (N * T) // NCORES   # tokens per core
P = 128                   # partitions / tokens per tile
NTILES = TOK // P         # 8
CC = F // P               # 4 contraction chunks of 128
MC = K // 1024            # 8 mega-chunks of 1024 classes

_BF16 = ml_dtypes.bfloat16
_FP8 = ml_dtypes.float8_e4m3
_cache: dict = {}


def build_program(has_bias: bool):
    """Build + compile the single-core Bass program (run SPMD on 8 cores)."""
    from concourse import bacc
    import concourse.bass as bass
    import concourse.tile as tile
    import concourse.mybir as mybir

    dt = mybir.dt
    alu = mybir.AluOpType
    act = mybir.ActivationFunctionType

    nc = bacc.Bacc(
        "TRN2", target_bir_lowering=False, debug=False, num_devices=NCORES
    )

    ctxT = nc.dram_tensor("ctxT", [F, TOK], dt.float8e4, kind="ExternalInput").ap()
    ctx = nc.dram_tensor("ctx", [TOK, F], dt.bfloat16, kind="ExternalInput").ap()
    featsT = nc.dram_tensor("featsT", [F, TOK], dt.bfloat16, kind="ExternalInput").ap()
    w = nc.dram_tensor("w", [F, K], dt.float8e4, kind="ExternalInput").ap()
    wt = nc.dram_tensor("wt", [K, F], dt.bfloat16, kind="ExternalInput").ap()
    # codebook.T replicated 8x along the contraction dim (row g*16+v = cb[:,v])
    cbt8 = nc.dram_tensor("cbt8", [P, K], dt.bfloat16, kind="ExternalInput").ap()
    # proj replicated 8x along its output dim (col g*16+v = proj[:,v])
    proj8 = nc.dram_tensor("proj8", [F, P], dt.bfloat16, kind="ExternalInput").ap()
    adjlen = nc.dram_tensor("adjlen", [P, 1], dt.float32, kind="ExternalInput").ap()
    tidx = nc.dram_tensor("tidx", [P, 1], dt.float32, kind="ExternalInput").ap()
    tidx_i = nc.dram_tensor("tidx_i", [P, 1], dt.int32, kind="ExternalInput").ap()
    if has_bias:
        brow = nc.dram_tensor("brow", [1, K], dt.bfloat16, kind="ExternalInput").ap()
        bcol = nc.dram_tensor("bcol", [K, 1], dt.float32, kind="ExternalInput").ap()
    out2 = nc.dram_tensor("out2", [2, 1], dt.float32, kind="ExternalOutput").ap()
    # DRAM staging for the two-level argmax: row (tok*MC + mc) holds that
    # token's mc-th 1024-wide score chunk (bf16).
    stage = nc.dram_tensor("scstage", [TOK * MC, 1024], dt.bfloat16).ap()
    stage_v = stage.rearrange("(t m) k -> t m k", m=MC)

    with tile.TileContext(nc) as tc:
        with (
            tc.tile_pool(name="singles", bufs=1) as singles,
            tc.tile_pool(name="work", bufs=3) as work,
            tc.tile_pool(name="stg", bufs=6) as stg,
            tc.tile_pool(name="sc_ps", bufs=2, space="PSUM") as sc_ps_pool,
            tc.tile_pool(name="lg_ps", bufs=2, space="PSUM") as lg_ps_pool,
        ):
            # ---- resident SBUF tensors ----
            w_sb = singles.tile([P, CC, K], dt.float8e4)
            ctxT_sb = singles.tile([P, CC, TOK], dt.float8e4)
            featsT_sb = singles.tile([P, CC, TOK], dt.bfloat16)
            ctx_sb = singles.tile([P, NTILES, F], dt.bfloat16)
            cbt8_sb = singles.tile([P, K], dt.bfloat16)
            proj8_sb = singles.tile([P, CC, P], dt.bfloat16)
            fT8_sb = singles.tile([P, TOK], dt.bfloat16)
            adjlen_sb = singles.tile([P, 1], dt.float32)
            tidx_sb = singles.tile([P, 1], dt.float32)
            tidxi_sb = singles.tile([P, 1], dt.int32)
            ones_sb = singles.tile([P, 1], dt.float32)
            warm_sb = singles.tile([P, 512], dt.bfloat16)
            exp_scr = singles.tile([P, 1024], dt.bfloat16)
            dot_scr = singles.tile([P, F], dt.bfloat16)
            nll_all = singles.tile([P, NTILES], dt.float32)
            cnt_all = singles.tile([P, NTILES], dt.float32)
            s_all = singles.tile([P, NTILES], dt.float32)
            lt_all = singles.tile([P, NTILES], dt.float32)
            logs_all = singles.tile([P, NTILES], dt.float32)
            stack2 = singles.tile([P, 2], dt.float32)
            out_sb = singles.tile([2, 1], dt.float32)

            # PE warm-up: matmuls on zeroed SBUF with no DMA dependency keep
            # the PE busy from t=0 so the HAM clock-gate opens to 2.4 GHz
            # while the input DMAs stream in.
            nc.vector.memset(warm_sb[:, :], 0.0)
            def emit_warm_mm(n=1):
                for _ in range(n):
                    wz = sc_ps_pool.tile([P, 1024], dt.float32, tag="sp", name="wz")
                    nc.tensor.matmul(
                        out=wz[:, 0:512], lhsT=warm_sb[:, 0:P], rhs=warm_sb[:, :],
                        start=True, stop=True,
                    )

            emit_warm_mm(24)

            # Startup loads. The sync queue is reserved for the per-chunk
            # score staging DMAs; the scalar queue's issues all complete
            # before the first exp (logits(0) runs in phase 0, after the
            # load window), so ACT is never delayed in the steady state.
            for cc in range(CC):
                nc.scalar.dma_start(out=featsT_sb[:, cc, :], in_=featsT[cc * P:(cc + 1) * P, :])
            # W in per-mega-chunk slices so the first logits matmul can start
            # after ~1 MB instead of the full 8 MB. Issue time is split
            # between the scalar queue (idle until tile 0's exps) and the
            # sync queue (ahead of the staging stores, which are first
            # consumed by the chainA(0) gather at loop j=1) so neither
            # queue eats the full ~20us of descriptor generation.
            for g in range(MC // 2):
                for cc in range(CC):
                    nc.scalar.dma_start(
                        out=w_sb[:, cc, g * 1024:(g + 1) * 1024],
                        in_=w[cc * P:(cc + 1) * P, g * 1024:(g + 1) * 1024],
                    )
            for cc in range(CC):
                nc.gpsimd.dma_start(out=proj8_sb[:, cc, :], in_=proj8[cc * P:(cc + 1) * P, :])
            # cbt8 in K-halves so the first scores mega can start early
            nc.gpsimd.dma_start(out=cbt8_sb[:, 0:K // 2], in_=cbt8[:, 0:K // 2])
            for cc in range(CC):
                nc.gpsimd.dma_start(out=ctxT_sb[:, cc, :], in_=ctxT[cc * P:(cc + 1) * P, :])
            nc.gpsimd.dma_start(out=cbt8_sb[:, K // 2:], in_=cbt8[:, K // 2:])
            for g in range(MC // 2, MC):
                for cc in range(CC):
                    nc.gpsimd.dma_start(
                        out=w_sb[:, cc, g * 1024:(g + 1) * 1024],
                        in_=w[cc * P:(cc + 1) * P, g * 1024:(g + 1) * 1024],
                    )
            for j in range(NTILES):
                nc.gpsimd.dma_start(out=ctx_sb[:, j, :], in_=ctx[j * P:(j + 1) * P, :])
            nc.gpsimd.dma_start(out=adjlen_sb[:, :], in_=adjlen[:, :])
            nc.gpsimd.dma_start(out=tidx_sb[:, :], in_=tidx[:, :])
            nc.gpsimd.dma_start(out=tidxi_sb[:, :], in_=tidx_i[:, :])
            nc.vector.memset(ones_sb[:, :], 1.0)

            if has_bias:
                onesrow_sb = singles.tile([1, P], dt.bfloat16)
                brow_sb = singles.tile([1, K], dt.bfloat16)
                nc.vector.memset(onesrow_sb[:, :], 1.0)
                nc.gpsimd.dma_start(out=brow_sb[:, :], in_=brow[:, :])

            # ---- fT8 = ((feats @ proj).T replicated 8x) : (128, TOK), bf16 ----
            fT8_ps = lg_ps_pool.tile([P, TOK], dt.float32, tag="lp")
            for h in range(TOK // 512):
                for cc in range(CC):
                    nc.tensor.matmul(
                        out=fT8_ps[:, h * 512:(h + 1) * 512],
                        lhsT=proj8_sb[:, cc, :],
                        rhs=featsT_sb[:, cc, h * 512:(h + 1) * 512],
                        start=(cc == 0),
                        stop=(cc == CC - 1),
                    )
            nc.vector.tensor_copy(out=fT8_sb[:, :], in_=fT8_ps[:, :])

            # ---- software-pipelined main loop ----
            st = {}  # per-tile live tiles: cmA/m1/mc8/scrow/widx/wrow

            def emit_scores_mega(t, mc):
                """One 1024-wide scores mega-chunk: two full-contraction
                matmuls into one PSUM tile, a single fused copy+max, then DMA
                the chunk to DRAM."""
                tsl = slice(t * P, (t + 1) * P)
                s = st.setdefault(t, {})
                if mc == 0:
                    s["cmA"] = work.tile([P, MC], dt.float32, tag="cma", name=f"cma{t}", bufs=4)
                if mc % 2 == 0:
                    # one staging tile covers two megas -> one coalesced DMA
                    s["sstg"] = stg.tile([P, 2, 1024], dt.bfloat16, tag="sstg",
                                         name=f"sstg{t}_{mc}")
                sstg = s["sstg"]
                sp = sc_ps_pool.tile([P, 1024], dt.float32, tag="sp")
                for h in range(2):
                    nc.tensor.matmul(
                        out=sp[:, h * 512:(h + 1) * 512],
                        lhsT=fT8_sb[:, tsl],
                        rhs=cbt8_sb[:, mc * 1024 + h * 512:mc * 1024 + (h + 1) * 512],
                        start=True,
                        stop=True,
                    )
                nc.vector.tensor_scalar(
                    out=sstg[:, mc % 2, :],
                    in0=sp[:, :],
                    scalar1=0.0,
                    scalar2=None,
                    op0=alu.add,
                    op1=alu.max,
                    accum_out=s["cmA"][:, mc:mc + 1],
                )
                if mc % 2 == 1:
                    nc.sync.dma_start(out=stage_v[tsl, mc - 1:mc + 1, :],
                                      in_=sstg[:, :, :])

            def emit_chainA(t):
                """Level-1 argmax over chunk maxes (DVE) + row index math and
                the score-chunk gather issue (Pool)."""
                s = st[t]
                cm = s["cmA"]
                m1 = work.tile([P, 1], dt.float32, tag="m1", name=f"m1_{t}")
                nc.vector.tensor_reduce(
                    out=m1[:, :], in_=cm[:, :], axis=mybir.AxisListType.X, op=alu.max
                )
                m8 = work.tile([P, 8], dt.float32, tag="m8", name=f"m8_{t}")
                nc.vector.tensor_copy(out=m8[:, :], in_=m1[:, 0:1].to_broadcast([P, 8]))
                mc8 = work.tile([P, 8], dt.uint32, tag="mc8", name=f"mc8_{t}")
                nc.vector.max_index(mc8[:, :], m8[:, :], cm[:, :])
                rowid = work.tile([P, 1], dt.int32, tag="rowid", name=f"rid{t}")
                _eng_idx = nc.gpsimd if _GPS_IDX else nc.vector
                _eng_idx.tensor_scalar(
                    out=rowid[:, :], in0=tidxi_sb[:, :],
                    scalar1=float(MC), scalar2=float(t * P * MC),
                    op0=alu.mult, op1=alu.add,
                )
                _eng_idx.tensor_tensor(
                    out=rowid[:, :], in0=rowid[:, :],
                    in1=mc8[:, 0:1].bitcast(dt.int32), op=alu.add,
                )
                scrow = work.tile([P, 1024], dt.bfloat16, tag="scrow",
                                  name=f"scrow{t}")
                nc.gpsimd.indirect_dma_start(
                    out=scrow[:, :],
                    out_offset=None,
                    in_=stage[:, :],
                    in_offset=bass.IndirectOffsetOnAxis(ap=rowid[:, 0:1], axis=0),
                )
                s["m1"], s["mc8"], s["scrow"] = m1, mc8, scrow

            def emit_chainB(t, part):
                """Level-2 argmax within the gathered chunk (staged a full
                tile-phase ago) + the W_enc.T row gather. Split into parts so
                the DVE/Pool streams interleave with the per-mega work."""
                s = st[t]
                if part == 0:
                    m1b = work.tile([P, 1], dt.bfloat16, tag="m1b", name=f"m1b{t}")
                    nc.vector.tensor_copy(out=m1b[:, :], in_=s["m1"][:, :])
                    m8b = work.tile([P, 8], dt.bfloat16, tag="m8b", name=f"m8b{t}")
                    nc.vector.tensor_copy(out=m8b[:, :], in_=m1b[:, 0:1].to_broadcast([P, 8]))
                    s["m8b"] = m8b
                elif part == 1:
                    l2i = work.tile([P, 8], dt.uint32, tag="l2i", name=f"l2i{t}")
                    nc.vector.max_index(l2i[:, :], s["m8b"][:, :], s["scrow"][:, :])
                    s["l2i"] = l2i
                elif part == 2:
                    widx = work.tile([P, 1], dt.int32, tag="widx", name=f"widx{t}")
                    _eng_idx = nc.gpsimd if _GPS_IDX else nc.vector
                    _eng_idx.tensor_scalar(
                        out=widx[:, :], in0=s["mc8"][:, 0:1].bitcast(dt.int32),
                        scalar1=1024.0, scalar2=None, op0=alu.mult,
                    )
                    _eng_idx.tensor_tensor(
                        out=widx[:, :], in0=widx[:, :],
                        in1=s["l2i"][:, 0:1].bitcast(dt.int32), op=alu.add,
                    )
                    s["widx"] = widx
                elif part == 3:
                    wrow = work.tile([P, F], dt.bfloat16, tag="wrow", name=f"wrow{t}")
                    nc.gpsimd.indirect_dma_start(
                        out=wrow[:, :],
                        out_offset=None,
                        in_=wt[:, :],
                        in_offset=bass.IndirectOffsetOnAxis(ap=s["widx"][:, 0:1], axis=0),
                    )
                    s["wrow"] = wrow
                    if has_bias:
                        bg = work.tile([P, 1], dt.float32, tag="bg", name=f"bg{t}")
                        nc.gpsimd.indirect_dma_start(
                            out=bg[:, :],
                            out_offset=None,
                            in_=bcol[:, :],
                            in_offset=bass.IndirectOffsetOnAxis(ap=s["widx"][:, 0:1], axis=0),
                        )
                        s["bg"] = bg

            def emit_dot(t):
                """Target logit via dot(ctx_row, W_row) (gather landed during
                the previous tile phase)."""
                s = st[t]
                if _GPS_DOT2:
                    # Pool two-step: elementwise product, then accumulate.
                    nc.gpsimd.tensor_tensor(
                        out=dot_scr[:, :], in0=ctx_sb[:, t, :],
                        in1=s["wrow"][:, :], op=alu.mult,
                    )
                    nc.gpsimd.tensor_scalar(
                        out=dot_scr[:, :], in0=dot_scr[:, :],
                        scalar1=1.0, scalar2=None, op0=alu.mult, op1=alu.add,
                        accum_out=lt_all[:, t:t + 1],
                    )
                    if has_bias:
                        nc.gpsimd.tensor_add(
                            lt_all[:, t:t + 1], lt_all[:, t:t + 1], s["bg"][:, :]
                        )
                    del st[t]
                    return
                (nc.gpsimd if _GPS_DOT else nc.vector).scalar_tensor_tensor(
                    out=dot_scr[:, :],
                    in0=ctx_sb[:, t, :],
                    scalar=1.0,
                    in1=s["wrow"][:, :],
                    op0=alu.mult,
                    op1=alu.mult,
                    accum_out=lt_all[:, t:t + 1],
                )
                if has_bias:
                    nc.gpsimd.tensor_add(
                        lt_all[:, t:t + 1], lt_all[:, t:t + 1], s["bg"][:, :]
                    )
                del st[t]

            def emit_logits_group(j, g, sums):
                tsl = slice(j * P, (j + 1) * P)
                lp = lg_ps_pool.tile([P, 1024], dt.float32, tag="lp")
                for h in range(2):
                    hsl = slice(h * 512, (h + 1) * 512)
                    for cc2 in range(0, CC, 2):
                        nc.tensor.matmul(
                            out=lp[:, hsl],
                            lhsT=ctxT_sb[:, cc2:cc2 + 2, tsl],
                            rhs=w_sb[:, cc2:cc2 + 2, g * 1024 + h * 512:g * 1024 + (h + 1) * 512],
                            start=(cc2 == 0),
                            stop=(cc2 == CC - 2 and not has_bias),
                            perf_mode=mybir.MatmulPerfMode.DoubleRow,
                        )
                    if has_bias:
                        nc.tensor.matmul(
                            out=lp[:, hsl],
                            lhsT=onesrow_sb[:, :],
                            rhs=brow_sb[:, g * 1024 + h * 512:g * 1024 + (h + 1) * 512],
                            start=False,
                            stop=True,
                        )
                nc.scalar.activation(
                    out=exp_scr[:, :],
                    in_=lp[:, :],
                    func=act.Exp,
                    scale=1.0 / 64.0,
                    accum_out=sums[:, g:g + 1],
                )

            def emit_mask(j):
                # valid mask: (tidx - adjlen) < -128*j  <=>  j*128 + tidx < len - t_off
                (nc.gpsimd if _GPS_MASK else nc.vector).tensor_scalar(
                    out=cnt_all[:, j:j + 1],
                    in0=tidx_sb[:, :],
                    scalar1=adjlen_sb[:, 0:1],
                    scalar2=float(-(j * P)),
                    op0=alu.subtract,
                    op1=alu.is_lt,
                )

            # Prologue: scores for tiles 0-2 (DVE-paced — there is no logits
            # work to hide them behind yet) interleaved with tile 0's logits
            # groups, which start as soon as their W slices land. The steady
            # drip of matmuls keeps the HAM clock-gate warm through the whole
            # input-load window.
            sums0 = work.tile([P, MC], dt.float32, tag="sums", name="sums0")
            for g in range(MC):
                for t in range(3):
                    emit_scores_mega(t, g)
                emit_logits_group(0, g, sums0)
            nc.vector.tensor_reduce(
                out=s_all[:, 0:1], in_=sums0[:, :],
                axis=mybir.AxisListType.X, op=alu.add,
            )
            emit_mask(0)
            emit_chainA(0)

            for j in range(1, NTILES):
                if j >= 2:
                    emit_dot(j - 2)
                if j < NTILES - 1:
                    # chainB(j-1)'s L2 MAX_INDEX depends on the scrow gather
                    # issued one phase ago; spreading its parts over the
                    # early groups gives the DMA a few extra us to land so
                    # the in-order DVE stream never stalls on it (the stall
                    # slid every later tile's chain and built a ~27us tail).
                    emit_chainB(j - 1, 0)
                    emit_chainA(j)
                    if j == NTILES - 2:
                        emit_chainA(NTILES - 1)
                else:
                    for part in range(4):
                        emit_chainB(j - 1, part)
                    for part in range(4):
                        emit_chainB(NTILES - 1, part)

                # logits (PE) + exp/row-sum (ACT), with tile j+2's scores
                # matmuls interleaved between groups
                sums = work.tile([P, MC], dt.float32, tag="sums", name=f"sums{j}")
                for g in range(MC):
                    if 3 <= j + 2 < NTILES:
                        emit_scores_mega(j + 2, g)
                    if j < NTILES - 1 and 1 <= g <= 3:
                        emit_chainB(j - 1, g)
                    if j == NTILES - 1:
                        if g == 3:
                            emit_dot(j - 1)
                        elif g == 6:
                            emit_dot(j)
                    emit_logits_group(j, g, sums)

                nc.vector.tensor_reduce(
                    out=s_all[:, j:j + 1], in_=sums[:, :],
                    axis=mybir.AxisListType.X, op=alu.add,
                )
                emit_mask(j)

            # ---- epilogue: one Ln for all tiles (avoids per-tile ACT
            # table-set switches between Exp and Ln), nll assembly, then the
            # partition reduction via ones-matmul ----
            nc.scalar.activation(out=logs_all[:, :], in_=s_all[:, :], func=act.Ln)
            nc.vector.tensor_sub(nll_all[:, :], logs_all[:, :], lt_all[:, :])
            nc.vector.tensor_mul(nll_all[:, :], nll_all[:, :], cnt_all[:, :])
            nc.vector.tensor_reduce(
                out=stack2[:, 0:1], in_=nll_all[:, :], axis=mybir.AxisListType.X,
                op=alu.add,
            )
            nc.vector.tensor_reduce(
                out=stack2[:, 1:2], in_=cnt_all[:, :], axis=mybir.AxisListType.X,
                op=alu.add,
            )
            fin_ps = sc_ps_pool.tile([2, 1], dt.float32, tag="sp")
            nc.tensor.matmul(
                out=fin_ps[:, :], lhsT=stack2[:, :], rhs=ones_sb[:, :],
                start=True, stop=True,
            )
            nc.vector.tensor_copy(out=out_sb[:, :], in_=fin_ps[:, :])
            nc.sync.dma_start(out=out2[:, :], in_=out_sb[:, :])

    nc.compile()
    return nc


def _get_program(has_bias: bool):
    if has_bias not in _cache:
        _cache[has_bias] = build_program(has_bias)
    return _cache[has_bias]


def make_in_maps(feats, context, lens, proj_matrix, codebook, W_enc, b_enc,
                 has_bias):
    """Shard + lay out the full inputs into per-core input maps."""
    feats_f = np.ascontiguousarray(feats).reshape(N * T, F)
    ctx_f = np.ascontiguousarray(context).reshape(N * T, F)
    w_f8 = (W_enc * 64.0).astype(_FP8)
    wt_bf = np.ascontiguousarray(W_enc.T).astype(_BF16)
    cbt8_bf = np.ascontiguousarray(np.tile(codebook.T.astype(_BF16), (8, 1)))
    proj8_bf = np.ascontiguousarray(np.tile(proj_matrix, (1, 8))).astype(_BF16)
    tidx_a = np.arange(P, dtype=np.float32).reshape(P, 1)
    tidx_ia = np.arange(P, dtype=np.int32).reshape(P, 1)

    in_maps = []
    for c in range(NCORES):
        sl = slice(c * TOK, (c + 1) * TOK)
        ctxs = ctx_f[sl]
        featss = feats_f[sl]
        n_idx = (c * TOK) // T
        t_off = (c * TOK) % T
        adj = np.full((P, 1), float(int(lens[n_idx]) - t_off), dtype=np.float32)
        m = {
            "ctxT": np.ascontiguousarray(ctxs.T).astype(_FP8),
            "ctx": ctxs.astype(_BF16),
            "featsT": np.ascontiguousarray(featss.T).astype(_BF16),
            "w": w_f8,
            "wt": wt_bf,
            "cbt8": cbt8_bf,
            "proj8": proj8_bf,
            "adjlen": adj,
            "tidx": tidx_a,
            "tidx_i": tidx_ia,
        }
        if has_bias:
            m["brow"] = np.ascontiguousarray(b_enc * 64.0).reshape(1, K).astype(_BF16)
            m["bcol"] = np.ascontiguousarray(b_enc).reshape(K, 1).astype(np.float32)
        in_maps.append(m)
    return in_maps


def kernel(feats, context, lens, proj_matrix, codebook, W_enc, b_enc,
           _want_results=False, _trace=False):
    from concourse.bass_utils import run_bass_kernel_spmd

    has_bias = bool(np.any(np.asarray(b_enc) != 0))
    nc = _get_program(has_bias)
    in_maps = make_in_maps(feats, context, lens, proj_matrix, codebook, W_enc,
                           b_enc, has_bias)
    res = run_bass_kernel_spmd(
        nc, in_maps, list(range(NCORES)), trace=_trace,
        trace_cores=list(range(NCORES)) if _trace else None,
    )
    num = sum(float(r["out2"][0, 0]) for r in res.results)
    cnt = sum(float(r["out2"][1, 0]) for r in res.results)
    loss = np.array(np.float32(num / max(cnt, 1.0)))
    if _want_results:
        return loss, res
    return loss


if __name__ == "__main__":
    import jax
    cpu = jax.devices("cpu")[0]
    import reference

    with jax.default_device(cpu):
        inputs = reference.setup_inputs()
        inputs = {k: np.asarray(v) for k, v in inputs.items()}
        expected = float(np.asarray(reference.reference(**inputs)))
    loss = float(kernel(**inputs))
    rel = abs(loss - expected) / max(abs(expected), 1e-30)
    print(f"expected {expected} got {loss} rel {rel:.3e}")


# revision 21
# speedup vs baseline: 1.1845x; 1.0367x over previous
"""Trainium2 Bass kernel for nn_BestRqLossNetwork (best-RQ masked-prediction loss).

Math (per the reference):
    logits  = context @ W_enc + b_enc                      # (N,T,K)
    targets = argmin_k ||normalize(feats @ proj) - cb_k||  # == argmax_k (feats@proj)·cb_k
                                                           #    (cb rows unit-norm, row norm > 0)
    loss    = mean over valid (t < lens[n]) of CE(logits, targets)

Distribution: data-parallel over the 8192 (n, t) positions — 1024 consecutive
tokens per core (each core's slab lies inside one sequence since T = 2*1024).
Weights (W_enc, codebook, proj) are replicated. Each core returns its local
(sum_nll, valid_count); the host sums the 16 scalars and divides.

Per-core pipeline, per 128-token tile (tokens on partitions):
  PE   : scores = fT8.T @ cbt8 — both replicated 8x along the contraction so
         all 128 PE rows are active (HAM keeps the clock-gate at 8/8; with
         16-row scores matmuls interleaved the PE sat at 4/8 = 1.2 GHz for
         the whole steady state). The 8x score scale is a power of two:
         argmax and bf16 rounding are unaffected.
         logits = ctxT.T @ W (fp8 DoubleRow) into 1024-wide PSUM groups.
  ACT  : exp with row-sum accumulation (logsumexp without max subtraction:
         |logits| <= ~6 so exp cannot overflow). One deferred Ln at the end.
         The scalar queue is left empty so ACT only ever runs exps.
  DVE  : fused PSUM->SBUF copy + per-1024-chunk max (tensor_scalar accum) —
         the only full-K touch DVE pays (any second pass or shaped reduce
         costs the same ~1.2ns/elem again); two-level argmax (MAX_INDEX over
         the 8 chunk maxes -> indirect-DMA gather of the winning 1024-chunk
         from a DRAM staging buffer -> MAX_INDEX within it).
  Pool : all index arithmetic, the valid-count mask, and the target-logit
         dot(ctx_row, gathered W_enc.T row) — keeps DVE under the PE's phase
         time so the scores PSUM never backpressures the PE.

Scheduling: engines execute their streams IN ORDER; emission is a uniform
software pipeline (phase j = logits(j) + scores(j+1)) with one tile-phase of
latency cover for each staging store -> gather -> consume hop:
  pre:     scores(0) + chainA(0)
  phase j: logits(j) groups interleaved with scores(j+1) megas;
           chainB(j) spread over early groups (L2 argmax + W-row gather);
           dot(j-1) at g=6; chainA(j+1) at the end (L1 argmax + score-chunk
           gather issue).
A few warm-up matmuls on zeroed SBUF run at the very start so the PE's HAM
clock-gate reaches 2.4 GHz before the real work arrives.
"""

import os
import numpy as np
import ml_dtypes

_GPS_IDX = os.environ.get("V4_GPS_IDX", "1") == "1"
# Pool scalar_tensor_tensor fails walrus codegen — keep the dot on DVE.
_GPS_DOT = os.environ.get("V4_GPS_DOT", "0") == "1"
_GPS_MASK = os.environ.get("V4_GPS_MASK", "1") == "1"
_GPS_DOT2 = os.environ.get("V4_GPS_DOT2", "0") == "1"

N, T, F, V, K = 4, 2048, 512, 16, 8192
NCORES = 8
TOK = (N * T) // NCORES   # tokens per core
P = 128                   # partitions / tokens per tile
NTILES = TOK // P         # 8
CC = F // P               # 4 contraction chunks of 128
MC = K // 1024            # 8 mega-chunks of 1024 classes

_BF16 = ml_dtypes.bfloat16
_FP8 = ml_dtypes.float8_e4m3
_cache: dict = {}


def build_program(has_bias: bool):
    """Build + compile the single-core Bass program (run SPMD on 8 cores)."""
    from concourse import bacc
    import concourse.bass as bass
    import concourse.tile as tile
    import concourse.mybir as mybir

    dt = mybir.dt
    alu = mybir.AluOpType
    act = mybir.ActivationFunctionType

    nc = bacc.Bacc(
        "TRN2", target_bir_lowering=False, debug=False, num_devices=NCORES
    )

    ctxT = nc.dram_tensor("ctxT", [F, TOK], dt.float8e4, kind="ExternalInput").ap()
    ctx = nc.dram_tensor("ctx", [TOK, F], dt.bfloat16, kind="ExternalInput").ap()
    featsT = nc.dram_tensor("featsT", [F, TOK], dt.bfloat16, kind="ExternalInput").ap()
    w = nc.dram_tensor("w", [F, K], dt.float8e4, kind="ExternalInput").ap()
    wt = nc.dram_tensor("wt", [K, F], dt.bfloat16, kind="ExternalInput").ap()
    # codebook.T replicated 8x along the contraction dim (row g*16+v = cb[:,v])
    cbt8 = nc.dram_tensor("cbt8", [P, K], dt.bfloat16, kind="ExternalInput").ap()
    # proj replicated 8x along its output dim (col g*16+v = proj[:,v])
    proj8 = nc.dram_tensor("proj8", [F, P], dt.bfloat16, kind="ExternalInput").ap()
    adjlen = nc.dram_tensor("adjlen", [P, 1], dt.float32, kind="ExternalInput").ap()
    tidx = nc.dram_tensor("tidx", [P, 1], dt.float32, kind="ExternalInput").ap()
    tidx_i = nc.dram_tensor("tidx_i", [P, 1], dt.int32, kind="ExternalInput").ap()
    if has_bias:
        brow = nc.dram_tensor("brow", [1, K], dt.bfloat16, kind="ExternalInput").ap()
        bcol = nc.dram_tensor("bcol", [K, 1], dt.float32, kind="ExternalInput").ap()
    out2 = nc.dram_tensor("out2", [2, 1], dt.float32, kind="ExternalOutput").ap()
    # DRAM staging for the two-level argmax: row (tok*MC + mc) holds that
    # token's mc-th 1024-wide score chunk (bf16).
    stage = nc.dram_tensor("scstage", [TOK * MC, 1024], dt.bfloat16).ap()
    stage_v = stage.rearrange("(t m) k -> t m k", m=MC)

    with tile.TileContext(nc) as tc:
        with (
            tc.tile_pool(name="singles", bufs=1) as singles,
            tc.tile_pool(name="work", bufs=3) as work,
            tc.tile_pool(name="stg", bufs=6) as stg,
            tc.tile_pool(name="sc_ps", bufs=2, space="PSUM") as sc_ps_pool,
            tc.tile_pool(name="lg_ps", bufs=2, space="PSUM") as lg_ps_pool,
        ):
            # ---- resident SBUF tensors ----
            w_sb = singles.tile([P, CC, K], dt.float8e4)
            ctxT_sb = singles.tile([P, CC, TOK], dt.float8e4)
            featsT_sb = singles.tile([P, CC, TOK], dt.bfloat16)
            ctx_sb = singles.tile([P, NTILES, F], dt.bfloat16)
            cbt8_sb = singles.tile([P, K], dt.bfloat16)
            proj8_sb = singles.tile([P, CC, P], dt.bfloat16)
            fT8_sb = singles.tile([P, TOK], dt.bfloat16)
            adjlen_sb = singles.tile([P, 1], dt.float32)
            tidx_sb = singles.tile([P, 1], dt.float32)
            tidxi_sb = singles.tile([P, 1], dt.int32)
            ones_sb = singles.tile([P, 1], dt.float32)
            warm_sb = singles.tile([P, 512], dt.bfloat16)
            exp_scr = singles.tile([P, 1024], dt.bfloat16)
            dot_scr = singles.tile([P, F], dt.bfloat16)
            nll_all = singles.tile([P, NTILES], dt.float32)
            cnt_all = singles.tile([P, NTILES], dt.float32)
            s_all = singles.tile([P, NTILES], dt.float32)
            lt_all = singles.tile([P, NTILES], dt.float32)
            logs_all = singles.tile([P, NTILES], dt.float32)
            stack2 = singles.tile([P, 2], dt.float32)
            out_sb = singles.tile([2, 1], dt.float32)

            # PE warm-up: matmuls on zeroed SBUF with no DMA dependency keep
            # the PE busy from t=0 so the HAM clock-gate opens to 2.4 GHz
            # while the input DMAs stream in.
            nc.vector.memset(warm_sb[:, :], 0.0)
            def emit_warm_mm(n=1):
                for _ in range(n):
                    wz = sc_ps_pool.tile([P, 1024], dt.float32, tag="sp", name="wz")
                    nc.tensor.matmul(
                        out=wz[:, 0:512], lhsT=warm_sb[:, 0:P], rhs=warm_sb[:, :],
                        start=True, stop=True,
                    )

            emit_warm_mm(10)

            # Startup loads. The sync queue is reserved for the per-chunk
            # score staging DMAs; the scalar queue's issues all complete
            # before the first exp (logits(0) runs in phase 0, after the
            # load window), so ACT is never delayed in the steady state.
            for cc in range(CC):
                nc.scalar.dma_start(out=featsT_sb[:, cc, :], in_=featsT[cc * P:(cc + 1) * P, :])
            # W in per-mega-chunk slices so the first logits matmul can start
            # after ~1 MB instead of the full 8 MB. Issue time is split
            # between the scalar queue (idle until tile 0's exps) and the
            # sync queue (ahead of the staging stores, which are first
            # consumed by the chainA(0) gather at loop j=1) so neither
            # queue eats the full ~20us of descriptor generation.
            for g in range(MC // 2):
                for cc in range(CC):
                    nc.scalar.dma_start(
                        out=w_sb[:, cc, g * 1024:(g + 1) * 1024],
                        in_=w[cc * P:(cc + 1) * P, g * 1024:(g + 1) * 1024],
                    )
            for cc in range(CC):
                nc.gpsimd.dma_start(out=proj8_sb[:, cc, :], in_=proj8[cc * P:(cc + 1) * P, :])
            # cbt8 in K-halves so the first scores mega can start early
            nc.gpsimd.dma_start(out=cbt8_sb[:, 0:K // 2], in_=cbt8[:, 0:K // 2])
            for cc in range(CC):
                nc.gpsimd.dma_start(out=ctxT_sb[:, cc, :], in_=ctxT[cc * P:(cc + 1) * P, :])
            nc.gpsimd.dma_start(out=cbt8_sb[:, K // 2:], in_=cbt8[:, K // 2:])
            for g in range(MC // 2, MC):
                for cc in range(CC):
                    nc.gpsimd.dma_start(
                        out=w_sb[:, cc, g * 1024:(g + 1) * 1024],
                        in_=w[cc * P:(cc + 1) * P, g * 1024:(g + 1) * 1024],
                    )
            for j in range(NTILES):
                nc.gpsimd.dma_start(out=ctx_sb[:, j, :], in_=ctx[j * P:(j + 1) * P, :])
            nc.gpsimd.dma_start(out=adjlen_sb[:, :], in_=adjlen[:, :])
            nc.gpsimd.dma_start(out=tidx_sb[:, :], in_=tidx[:, :])
            nc.gpsimd.dma_start(out=tidxi_sb[:, :], in_=tidx_i[:, :])
            nc.vector.memset(ones_sb[:, :], 1.0)

            if has_bias:
                onesrow_sb = singles.tile([1, P], dt.bfloat16)
                brow_sb = singles.tile([1, K], dt.bfloat16)
                nc.vector.memset(onesrow_sb[:, :], 1.0)
                nc.gpsimd.dma_start(out=brow_sb[:, :], in_=brow[:, :])

            # ---- fT8 = ((feats @ proj).T replicated 8x) : (128, TOK), bf16 ----
            fT8_ps = lg_ps_pool.tile([P, TOK], dt.float32, tag="lp")
            for h in range(TOK // 512):
                for cc in range(CC):
                    nc.tensor.matmul(
                        out=fT8_ps[:, h * 512:(h + 1) * 512],
                        lhsT=proj8_sb[:, cc, :],
                        rhs=featsT_sb[:, cc, h * 512:(h + 1) * 512],
                        start=(cc == 0),
                        stop=(cc == CC - 1),
                    )
            nc.vector.tensor_copy(out=fT8_sb[:, :], in_=fT8_ps[:, :])

            # ---- software-pipelined main loop ----
            st = {}  # per-tile live tiles: cmA/m1/mc8/scrow/widx/wrow

            def emit_scores_mega(t, mc):
                """One 1024-wide scores mega-chunk: two full-contraction
                matmuls into one PSUM tile, a single fused copy+max, then DMA
                the chunk to DRAM."""
                tsl = slice(t * P, (t + 1) * P)
                s = st.setdefault(t, {})
                if mc == 0:
                    s["cmA"] = work.tile([P, MC], dt.float32, tag="cma", name=f"cma{t}", bufs=4)
                if mc % 2 == 0:
                    # one staging tile covers two megas -> one coalesced DMA
                    s["sstg"] = stg.tile([P, 2, 1024], dt.bfloat16, tag="sstg",
                                         name=f"sstg{t}_{mc}")
                sstg = s["sstg"]
                sp = sc_ps_pool.tile([P, 1024], dt.float32, tag="sp")
                for h in range(2):
                    nc.tensor.matmul(
                        out=sp[:, h * 512:(h + 1) * 512],
                        lhsT=fT8_sb[:, tsl],
                        rhs=cbt8_sb[:, mc * 1024 + h * 512:mc * 1024 + (h + 1) * 512],
                        start=True,
                        stop=True,
                    )
                nc.vector.tensor_scalar(
                    out=sstg[:, mc % 2, :],
                    in0=sp[:, :],
                    scalar1=0.0,
                    scalar2=None,
                    op0=alu.add,
                    op1=alu.max,
                    accum_out=s["cmA"][:, mc:mc + 1],
                )
                if mc % 2 == 1:
                    nc.sync.dma_start(out=stage_v[tsl, mc - 1:mc + 1, :],
                                      in_=sstg[:, :, :])

            def emit_chainA(t):
                """Level-1 argmax over chunk maxes (DVE) + row index math and
                the score-chunk gather issue (Pool)."""
                s = st[t]
                cm = s["cmA"]
                m1 = work.tile([P, 1], dt.float32, tag="m1", name=f"m1_{t}")
                nc.vector.tensor_reduce(
                    out=m1[:, :], in_=cm[:, :], axis=mybir.AxisListType.X, op=alu.max
                )
                mc8 = work.tile([P, 8], dt.uint32, tag="mc8", name=f"mc8_{t}")
                nc.vector.max_index(mc8[:, :], m1[:, 0:1].to_broadcast([P, 8]), cm[:, :])
                rowid = work.tile([P, 1], dt.int32, tag="rowid", name=f"rid{t}")
                _eng_idx = nc.gpsimd if _GPS_IDX else nc.vector
                _eng_idx.tensor_scalar(
                    out=rowid[:, :], in0=tidxi_sb[:, :],
                    scalar1=float(MC), scalar2=float(t * P * MC),
                    op0=alu.mult, op1=alu.add,
                )
                _eng_idx.tensor_tensor(
                    out=rowid[:, :], in0=rowid[:, :],
                    in1=mc8[:, 0:1].bitcast(dt.int32), op=alu.add,
                )
                scrow = work.tile([P, 1024], dt.bfloat16, tag="scrow",
                                  name=f"scrow{t}")
                nc.gpsimd.indirect_dma_start(
                    out=scrow[:, :],
                    out_offset=None,
                    in_=stage[:, :],
                    in_offset=bass.IndirectOffsetOnAxis(ap=rowid[:, 0:1], axis=0),
                )
                s["m1"], s["mc8"], s["scrow"] = m1, mc8, scrow

            def emit_chainB(t, part):
                """Level-2 argmax within the gathered chunk (staged a full
                tile-phase ago) + the W_enc.T row gather. Split into parts so
                the DVE/Pool streams interleave with the per-mega work."""
                s = st[t]
                if part == 0:
                    m1b = work.tile([P, 1], dt.bfloat16, tag="m1b", name=f"m1b{t}")
                    nc.vector.tensor_copy(out=m1b[:, :], in_=s["m1"][:, :])
                    s["m1b"] = m1b
                elif part == 1:
                    l2i = work.tile([P, 8], dt.uint32, tag="l2i", name=f"l2i{t}")
                    nc.vector.max_index(
                        l2i[:, :], s["m1b"][:, 0:1].to_broadcast([P, 8]),
                        s["scrow"][:, :])
                    s["l2i"] = l2i
                elif part == 2:
                    widx = work.tile([P, 1], dt.int32, tag="widx", name=f"widx{t}")
                    _eng_idx = nc.gpsimd if _GPS_IDX else nc.vector
                    _eng_idx.tensor_scalar(
                        out=widx[:, :], in0=s["mc8"][:, 0:1].bitcast(dt.int32),
                        scalar1=1024.0, scalar2=None, op0=alu.mult,
                    )
                    _eng_idx.tensor_tensor(
                        out=widx[:, :], in0=widx[:, :],
                        in1=s["l2i"][:, 0:1].bitcast(dt.int32), op=alu.add,
                    )
                    s["widx"] = widx
                elif part == 3:
                    wrow = work.tile([P, F], dt.bfloat16, tag="wrow", name=f"wrow{t}")
                    nc.gpsimd.indirect_dma_start(
                        out=wrow[:, :],
                        out_offset=None,
                        in_=wt[:, :],
                        in_offset=bass.IndirectOffsetOnAxis(ap=s["widx"][:, 0:1], axis=0),
                    )
                    s["wrow"] = wrow
                    if has_bias:
                        bg = work.tile([P, 1], dt.float32, tag="bg", name=f"bg{t}")
                        nc.gpsimd.indirect_dma_start(
                            out=bg[:, :],
                            out_offset=None,
                            in_=bcol[:, :],
                            in_offset=bass.IndirectOffsetOnAxis(ap=s["widx"][:, 0:1], axis=0),
                        )
                        s["bg"] = bg

            def emit_dot(t):
                """Target logit via dot(ctx_row, W_row) (gather landed during
                the previous tile phase)."""
                s = st[t]
                if _GPS_DOT2:
                    # Pool two-step: elementwise product, then accumulate.
                    nc.gpsimd.tensor_tensor(
                        out=dot_scr[:, :], in0=ctx_sb[:, t, :],
                        in1=s["wrow"][:, :], op=alu.mult,
                    )
                    nc.gpsimd.tensor_scalar(
                        out=dot_scr[:, :], in0=dot_scr[:, :],
                        scalar1=1.0, scalar2=None, op0=alu.mult, op1=alu.add,
                        accum_out=lt_all[:, t:t + 1],
                    )
                    if has_bias:
                        nc.gpsimd.tensor_add(
                            lt_all[:, t:t + 1], lt_all[:, t:t + 1], s["bg"][:, :]
                        )
                    del st[t]
                    return
                (nc.gpsimd if _GPS_DOT else nc.vector).scalar_tensor_tensor(
                    out=dot_scr[:, :],
                    in0=ctx_sb[:, t, :],
                    scalar=1.0,
                    in1=s["wrow"][:, :],
                    op0=alu.mult,
                    op1=alu.mult,
                    accum_out=lt_all[:, t:t + 1],
                )
                if has_bias:
                    nc.gpsimd.tensor_add(
                        lt_all[:, t:t + 1], lt_all[:, t:t + 1], s["bg"][:, :]
                    )
                del st[t]

            def emit_logits_group(j, g, sums):
                tsl = slice(j * P, (j + 1) * P)
                lp = lg_ps_pool.tile([P, 1024], dt.float32, tag="lp")
                for h in range(2):
                    hsl = slice(h * 512, (h + 1) * 512)
                    for cc2 in range(0, CC, 2):
                        nc.tensor.matmul(
                            out=lp[:, hsl],
                            lhsT=ctxT_sb[:, cc2:cc2 + 2, tsl],
                            rhs=w_sb[:, cc2:cc2 + 2, g * 1024 + h * 512:g * 1024 + (h + 1) * 512],
                            start=(cc2 == 0),
                            stop=(cc2 == CC - 2 and not has_bias),
                            perf_mode=mybir.MatmulPerfMode.DoubleRow,
                        )
                    if has_bias:
                        nc.tensor.matmul(
                            out=lp[:, hsl],
                            lhsT=onesrow_sb[:, :],
                            rhs=brow_sb[:, g * 1024 + h * 512:g * 1024 + (h + 1) * 512],
                            start=False,
                            stop=True,
                        )
                nc.scalar.activation(
                    out=exp_scr[:, :],
                    in_=lp[:, :],
                    func=act.Exp,
                    scale=1.0 / 64.0,
                    accum_out=sums[:, g:g + 1],
                )

            def emit_mask(j):
                # valid mask: (tidx - adjlen) < -128*j  <=>  j*128 + tidx < len - t_off
                (nc.gpsimd if _GPS_MASK else nc.vector).tensor_scalar(
                    out=cnt_all[:, j:j + 1],
                    in0=tidx_sb[:, :],
                    scalar1=adjlen_sb[:, 0:1],
                    scalar2=float(-(j * P)),
                    op0=alu.subtract,
                    op1=alu.is_lt,
                )

            # Prologue: scores for tiles 0-2 (DVE-paced — there is no logits
            # work to hide them behind yet) interleaved with tile 0's logits
            # groups, which start as soon as their W slices land. The steady
            # drip of matmuls keeps the HAM clock-gate warm through the whole
            # input-load window.
            sums0 = work.tile([P, MC], dt.float32, tag="sums", name="sums0")
            for g in range(MC):
                for t in range(3):
                    emit_scores_mega(t, g)
                emit_logits_group(0, g, sums0)
            nc.vector.tensor_reduce(
                out=s_all[:, 0:1], in_=sums0[:, :],
                axis=mybir.AxisListType.X, op=alu.add,
            )
            emit_mask(0)
            emit_chainA(0)

            for j in range(1, NTILES):
                if j >= 2:
                    emit_dot(j - 2)
                if j < NTILES - 1:
                    # chainB(j-1)'s L2 MAX_INDEX depends on the scrow gather
                    # issued one phase ago; spreading its parts over the
                    # early groups gives the DMA a few extra us to land so
                    # the in-order DVE stream never stalls on it (the stall
                    # slid every later tile's chain and built a ~27us tail).
                    emit_chainB(j - 1, 0)
                    emit_chainA(j)
                    if j == NTILES - 2:
                        emit_chainA(NTILES - 1)
                else:
                    for part in range(4):
                        emit_chainB(j - 1, part)
                    for part in range(4):
                        emit_chainB(NTILES - 1, part)

                # logits (PE) + exp/row-sum (ACT), with tile j+2's scores
                # matmuls interleaved between groups
                sums = work.tile([P, MC], dt.float32, tag="sums", name=f"sums{j}")
                for g in range(MC):
                    if 3 <= j + 2 < NTILES:
                        emit_scores_mega(j + 2, g)
                    if j < NTILES - 1 and 1 <= g <= 3:
                        emit_chainB(j - 1, g)
                    if j == NTILES - 1:
                        if g == 3:
                            emit_dot(j - 1)
                        elif g == 6:
                            emit_dot(j)
                    emit_logits_group(j, g, sums)

                nc.vector.tensor_reduce(
                    out=s_all[:, j:j + 1], in_=sums[:, :],
                    axis=mybir.AxisListType.X, op=alu.add,
                )
                emit_mask(j)

            # ---- epilogue: one Ln for all tiles (avoids per-tile ACT
            # table-set switches between Exp and Ln), nll assembly, then the
            # partition reduction via ones-matmul ----
            nc.scalar.activation(out=logs_all[:, :], in_=s_all[:, :], func=act.Ln)
            nc.vector.tensor_sub(nll_all[:, :], logs_all[:, :], lt_all[:, :])
            nc.vector.tensor_mul(nll_all[:, :], nll_all[:, :], cnt_all[:, :])
            nc.vector.tensor_reduce(
                out=stack2[:, 0:1], in_=nll_all[:, :], axis=mybir.AxisListType.X,
                op=alu.add,
            )
            nc.vector.tensor_reduce(
                out=stack2[:, 1:2], in_=cnt_all[:, :], axis=mybir.AxisListType.X,
                op=alu.add,
            )
            fin_ps = sc_ps_pool.tile([2, 1], dt.float32, tag="sp")
            nc.tensor.matmul(
                out=fin_ps[:, :], lhsT=stack2[:, :], rhs=ones_sb[:, :],
                start=True, stop=True,
            )
            nc.vector.tensor_copy(out=out_sb[:, :], in_=fin_ps[:, :])
            nc.sync.dma_start(out=out2[:, :], in_=out_sb[:, :])

    nc.compile()
    return nc


def _get_program(has_bias: bool):
    if has_bias not in _cache:
        _cache[has_bias] = build_program(has_bias)
    return _cache[has_bias]


def make_in_maps(feats, context, lens, proj_matrix, codebook, W_enc, b_enc,
                 has_bias):
    """Shard + lay out the full inputs into per-core input maps."""
    feats_f = np.ascontiguousarray(feats).reshape(N * T, F)
    ctx_f = np.ascontiguousarray(context).reshape(N * T, F)
    w_f8 = (W_enc * 64.0).astype(_FP8)
    wt_bf = np.ascontiguousarray(W_enc.T).astype(_BF16)
    cbt8_bf = np.ascontiguousarray(np.tile(codebook.T.astype(_BF16), (8, 1)))
    proj8_bf = np.ascontiguousarray(np.tile(proj_matrix, (1, 8))).astype(_BF16)
    tidx_a = np.arange(P, dtype=np.float32).reshape(P, 1)
    tidx_ia = np.arange(P, dtype=np.int32).reshape(P, 1)

    in_maps = []
    for c in range(NCORES):
        sl = slice(c * TOK, (c + 1) * TOK)
        ctxs = ctx_f[sl]
        featss = feats_f[sl]
        n_idx = (c * TOK) // T
        t_off = (c * TOK) % T
        adj = np.full((P, 1), float(int(lens[n_idx]) - t_off), dtype=np.float32)
        m = {
            "ctxT": np.ascontiguousarray(ctxs.T).astype(_FP8),
            "ctx": ctxs.astype(_BF16),
            "featsT": np.ascontiguousarray(featss.T).astype(_BF16),
            "w": w_f8,
            "wt": wt_bf,
            "cbt8": cbt8_bf,
            "proj8": proj8_bf,
            "adjlen": adj,
            "tidx": tidx_a,
            "tidx_i": tidx_ia,
        }
        if has_bias:
            m["brow"] = np.ascontiguousarray(b_enc * 64.0).reshape(1, K).astype(_BF16)
            m["bcol"] = np.ascontiguousarray(b_enc).reshape(K, 1).astype(np.float32)
        in_maps.append(m)
    return in_maps


def kernel(feats, context, lens, proj_matrix, codebook, W_enc, b_enc,
           _want_results=False, _trace=False):
    from concourse.bass_utils import run_bass_kernel_spmd

    has_bias = bool(np.any(np.asarray(b_enc) != 0))
    nc = _get_program(has_bias)
    in_maps = make_in_maps(feats, context, lens, proj_matrix, codebook, W_enc,
                           b_enc, has_bias)
    res = run_bass_kernel_spmd(
        nc, in_maps, list(range(NCORES)), trace=_trace,
        trace_cores=list(range(NCORES)) if _trace else None,
    )
    num = sum(float(r["out2"][0, 0]) for r in res.results)
    cnt = sum(float(r["out2"][1, 0]) for r in res.results)
    loss = np.array(np.float32(num / max(cnt, 1.0)))
    if _want_results:
        return loss, res
    return loss


if __name__ == "__main__":
    import jax
    cpu = jax.devices("cpu")[0]
    import reference

    with jax.default_device(cpu):
        inputs = reference.setup_inputs()
        inputs = {k: np.asarray(v) for k, v in inputs.items()}
        expected = float(np.asarray(reference.reference(**inputs)))
    loss = float(kernel(**inputs))
    rel = abs(loss - expected) / max(abs(expected), 1e-30)
    print(f"expected {expected} got {loss} rel {rel:.3e}")
